# revision 39
# baseline (speedup 1.0000x reference)
"""MHA Trainium2 kernel, 8 NeuronCores.

Problem: B=2, S=2048, D=2560, H=32 heads, HD=80, partial rotary RD=32,
causal attention, fp32 I/O.

Active path (v2, ~628us HW): hybrid sharding -- cores 0-3 take batch 0,
cores 4-7 batch 1; 8 heads per core. All matmul operands bf16 (PE runs
1 cycle/row, same as f32r, but half the DMA/SBUF), fp32 PSUM accumulate.

Per 512-token block, emitted interleaved:
  qk:  transposed-direct projection: stationary = weight tiles, moving =
       x^T tiles, so q/k land as [head_dim, tokens] with no PE transpose.
       Heads padded 80->96 rows (engine APs need 32-aligned partition
       starts; 96 makes every head-boundary piece legal). Bias added
       during the PSUM->SBUF distribution (chunked tensor_scalar_add);
       rope = PE swap-matmul (16-row block swap) + 3 wide DVE ops,
       delayed one group so the PE never waits on the staging chain.
  v:   natural layout [tokens, 80(+zeros+ones col at 96)] from a fully
       resident wv; the ones column accumulates the softmax denominator
       for free inside the PV matmul.
  at:  scoresT = kT.T @ qT (128-row padded stationaries enable fast
       weight load), exp on ACT (bf16 out), causal mask multiply only on
       diagonal tiles, columns trimmed to the valid causal range;
       scores->exp->PV software-pipelined 3 deep. Denominator row 96 ->
       reciprocal_approx_fast (via SBUF; the custom DVE op misreads at
       PSUM partition offsets) -> gpsimd partition_broadcast -> DVE
       normalize into packed 128-row ctx tiles.
  out: y^T partials [d, tokens], deferred one block and interleaved into
       the next block's attention so independent out matmuls fill the
       exp-latency stalls (keeps the PE HAM clock gate at 2.4 GHz).
Host: per batch, sum the 4 cores' fp32 partials, transpose, add bias.
"""
import sys
import os

sys.path.insert(0, "/opt/trn_rl_repo")

import numpy as np
from contextlib import ExitStack

import concourse.bacc as bacc
import concourse.tile as tile
import concourse.mybir as mybir
from concourse.bass_utils import run_bass_kernel_spmd
from concourse.masks import make_identity

F32 = mybir.dt.float32
F32R = mybir.dt.float32r

B, S, D = 2, 2048, 2560
H, HD = 32, 80
RD = 32
ROPE_BASE = 10000.0
N_CORES = 8


def rne11(x):
    """Round-to-nearest-even to 11 mantissa bits (matches HW f32r rounding)."""
    xi = np.ascontiguousarray(x, dtype=np.float32).view(np.uint32).astype(np.uint64)
    shift = np.uint64(12)
    bias = np.uint64((1 << 11) - 1)
    lsb = (xi >> shift) & np.uint64(1)
    xi = (xi + bias + lsb) >> shift << shift
    return xi.astype(np.uint32).view(np.float32)


def make_cfg(s=S, d=D, nh=H // N_CORES, hd=HD, rd=RD, b=B, qb=512, dt_mm="f32"):
    cfg = dict(s=s, d=d, nh=nh, hd=hd, rd=rd, b=b, qb=qb, dt_mm=dt_mm)
    cfg["n_st"] = s // 128
    cfg["n_kt"] = (d + 1 + 127) // 128
    cfg["d_aug"] = cfg["n_kt"] * 128
    cfg["nqb"] = s // qb
    cfg["ndiag"] = qb // 128
    cfg["n_dt"] = d // 128
    cfg["jq"] = nh * hd
    return cfg


# ---------------------------------------------------------------------------
# v2: batch x head-group hybrid sharding, bf16 operands, transposed-direct
# q/k projection, interleaved per-512-token-block emission.
# ---------------------------------------------------------------------------

BF = mybir.dt.bfloat16
NH2 = 8          # heads per core
TB = 512         # token block
NTT = S // TB    # 4 token tiles
KT2 = D // 128   # 20 contraction tiles for q,k
HDP = 96         # head rows padded to 96 (32-aligned piece boundaries)
NJT = NH2 * HDP // 128  # 6 j-tiles of 128 in padded j-space


def make_cfg_v2():
    return dict(ver=2)


def _qk_pieces(jt):
    """Rows of j-tile jt split at (padded) head boundaries: (r0, r1, h, hr0).

    All boundaries are multiples of 32 since HDP % 32 == 0.
    """
    out = []
    r = 0
    while r < 128:
        j = jt * 128 + r
        h, hr = j // HDP, j % HDP
        r1 = min(128, r + (HDP - hr))
        out.append((r, r1, h, hr))
        r = r1
    return out


def _heads_done(jt):
    """Heads fully staged after j-tile jt."""
    lo = (128 * jt) // HDP if jt > 0 else 0
    hi = (128 * (jt + 1)) // HDP
    return range(lo, hi)


def _part_chunks(src0, dst0, n):
    """Split a partition-range copy into engine-legal chunks.

    Engine APs must be naturally aligned in the partition dim: start 0 is
    unrestricted, start 64 allows <=64 rows, starts 32/96 allow <=32.
    """
    def maxsz(x):
        if x == 0:
            return 128
        return 64 if x % 64 == 0 else 32

    out = []
    r = 0
    while r < n:
        s, d = src0 + r, dst0 + r
        sz = min(maxsz(s), maxsz(d), n - r)
        out.append((s, d, sz))
        r += sz
    return out


def build_program_v2(cfg):
    EXP = mybir.ActivationFunctionType.Exp
    nc = bacc.Bacc(None, debug=False)

    xt_d = nc.declare_dram_parameter("xt", [NTT, 128, (KT2 + 1) * TB], BF,
                                     isOutput=False)
    wqk_d = nc.declare_dram_parameter("wqk", [2, NJT, 128, KT2 * 128], BF,
                                      isOutput=False)
    bqk_d = nc.declare_dram_parameter("bqk", [128, 2 * NJT], F32,
                                      isOutput=False)
    wv_d = nc.declare_dram_parameter("wv", [128, (KT2 + 1) * NH2 * HD], BF,
                                     isOutput=False)
    ow_d = nc.declare_dram_parameter("ow", [NJT, 128, D], BF, isOutput=False)
    cs_d = nc.declare_dram_parameter("cs", [2 * RD, S], BF, isOutput=False)
    masks_d = nc.declare_dram_parameter("masks", [128, 4 * TB], BF,
                                        isOutput=False)
    p32_d = nc.declare_dram_parameter("p32", [RD, RD], BF, isOutput=False)
    y_d = nc.declare_dram_parameter("y", [D, S], F32, isOutput=True)
    if cfg.get("dump"):
        dq_d = nc.declare_dram_parameter("dump_q", [HDP, TB], BF,
                                         isOutput=True)
        dk_d = nc.declare_dram_parameter("dump_k", [HDP, S], BF,
                                         isOutput=True)
        dv_d = nc.declare_dram_parameter("dump_v", [128, 16 * 97], BF,
                                         isOutput=True)
        dst_d = nc.declare_dram_parameter("dump_st", [128, TB], BF,
                                          isOutput=True)
        dctx_d = nc.declare_dram_parameter("dump_ctx", [128, TB], BF,
                                           isOutput=True)
        drb_d = nc.declare_dram_parameter("dump_rb", [HDP, TB], F32,
                                          isOutput=True)
        dpd_d = nc.declare_dram_parameter("dump_pd", [1, TB], F32,
                                          isOutput=True)

    with tile.TileContext(nc) as tc, ExitStack() as top:
        glob = top.enter_context(tc.tile_pool(name="glob", bufs=1))
        cs = glob.tile([2 * RD, S], BF)
        masks = glob.tile([128, 4 * TB], BF)
        bqk = glob.tile([128, 2 * NJT], F32)
        p32 = glob.tile([RD, RD], BF)
        owt = [glob.tile([128, D], BF, name=f"owt{jt}") for jt in range(NJT)]

        wv_sb = glob.tile([128, (KT2 + 1) * NH2 * HD], BF)
        wv3 = wv_sb.rearrange("p (k j) -> p k j", k=KT2 + 1)

        VC = 97  # v cols: 80 data + zeros + ones col at 96 (32-aligned)
        kT = [glob.tile([128, S], BF, name=f"kT{h}") for h in range(NH2)]
        for h in range(NH2):
            nc.vector.memset(kT[h][HDP:128, :], 0.0)
        vA = [glob.tile([128, 16 * VC], BF, name=f"vA{h}")
              for h in range(NH2)]
        vA3 = [v.rearrange("p (s c) -> p s c", s=16) for v in vA]
        for h in range(NH2):
            nc.vector.memset(vA3[h][:, :, HD:VC - 1], 0.0)
            nc.vector.memset(vA3[h][:, :, VC - 1:VC], 1.0)

        xpool = top.enter_context(tc.tile_pool(name="xs", bufs=1))
        wqkpool = top.enter_context(tc.tile_pool(name="wqks", bufs=3))
        qtpool = top.enter_context(tc.tile_pool(name="qt", bufs=1))
        rtmp = top.enter_context(tc.tile_pool(name="rtmp", bufs=2))
        ptpool = top.enter_context(tc.tile_pool(name="pt", bufs=5))
        rdpool = top.enter_context(tc.tile_pool(name="rd", bufs=1))
        rbpool = top.enter_context(tc.tile_pool(name="rb", bufs=2))
        chpool = top.enter_context(tc.tile_pool(name="ch", bufs=2))
        ctxpool = top.enter_context(tc.tile_pool(name="ctx", bufs=2))
        ypool = top.enter_context(tc.tile_pool(name="yst", bufs=2))
        Po = top.enter_context(tc.tile_pool(name="pso", bufs=1, space="PSUM"))

        def make_out_emitter(ctxt_cur, win_cur, tt_cur):
            def emit(dt_i):
                psy = Po.tile([128, TB], F32, tag="py",
                              name=f"psy{tt_cur}_{dt_i}")
                for jt in range(NJT):
                    nc.tensor.matmul(
                        psy, owt[jt][:, dt_i * 128:(dt_i + 1) * 128],
                        ctxt_cur[jt], start=(jt == 0), stop=(jt == NJT - 1))
                yt = ypool.tile([128, TB], F32, tag="y")
                nc.vector.tensor_copy(yt, psy)
                nc.sync.dma_start(
                    out=y_d[dt_i * 128:(dt_i + 1) * 128, win_cur], in_=yt)
            return emit

        pending_out = None  # [emit_fn, remaining dt list]

        for tt in range(NTT):
            win = slice(tt * TB, (tt + 1) * TB)
            xt = xpool.tile([128, (KT2 + 1) * TB], BF, tag="xt")
            for c0, c1 in ((0, 4), (4, 10), (10, 16), (16, KT2 + 1)):
                nc.sync.dma_start(out=xt[:, c0 * TB:c1 * TB],
                                  in_=xt_d[tt][:, c0 * TB:c1 * TB])
            xt3 = xt.rearrange("p (k t) -> p k t", k=KT2 + 1)

            qT = [qtpool.tile([128, TB], BF, tag=f"q{h}", name=f"qT{tt}_{h}")
                  for h in range(NH2)]
            if tt == 0:
                for h in range(NH2):
                    nc.vector.memset(qT[h][HDP:128, :], 0.0)

            # ---- q,k projection (transposed-direct) + rope ----
            with nc.named_scope(f"t{tt}_qk"), ExitStack() as pstk:
                P1 = pstk.enter_context(
                    tc.tile_pool(name=f"psqk{tt}", bufs=3, space="PSUM"))
                Psw = pstk.enter_context(
                    tc.tile_pool(name=f"psw{tt}", bufs=2, space="PSUM"))
                def do_rope(qk, heads):
                    for h in heads:
                        rot = (qT[h][0:RD, :] if qk == 0
                               else kT[h][0:RD, win])
                        sw = Psw.tile([RD, TB], F32, tag="sw")
                        nc.tensor.matmul(sw, p32, rot,
                                         start=True, stop=True)
                        tcos = rtmp.tile([RD, TB], BF, tag="tc")
                        nc.vector.tensor_mul(tcos, rot, cs[0:RD, win])
                        tsin = rtmp.tile([RD, TB], BF, tag="ts")
                        nc.vector.tensor_mul(tsin, sw, cs[RD:2 * RD, win])
                        nc.vector.tensor_add(rot, tcos, tsin)

                pending = None
                for qk in range(2):
                    for jt in range(NJT):
                        w = wqkpool.tile([128, KT2 * 128], BF, tag="wqk")
                        nc.sync.dma_start(out=w, in_=wqk_d[qk, jt])
                        if tt == 0 and qk == 0 and jt == 0:
                            nc.sync.dma_start(out=bqk, in_=bqk_d[:, :])
                            nc.sync.dma_start(out=cs, in_=cs_d[:, :])
                            nc.sync.dma_start(out=p32, in_=p32_d[:, :])
                        w3 = w.rearrange("p (k j) -> p k j", k=KT2)
                        ps = P1.tile([128, TB], F32, tag="ps")
                        for kt in range(KT2):
                            nc.tensor.matmul(ps, w3[:, kt, :], xt3[:, kt, :],
                                             start=(kt == 0),
                                             stop=(kt == KT2 - 1))
                        # rope for the previous group's completed heads --
                        # one group late so the PE never waits on the DVE
                        # staging chain
                        if pending:
                            do_rope(*pending)
                        bcol = qk * NJT + jt
                        for (r0, r1, h, hr0) in _qk_pieces(jt):
                            for (s0, d0, n) in _part_chunks(r0, hr0,
                                                            r1 - r0):
                                if qk == 0:
                                    dst = qT[h][d0:d0 + n, :]
                                else:
                                    dst = kT[h][d0:d0 + n, win]
                                nc.any.tensor_scalar_add(
                                    dst, ps[s0:s0 + n, :],
                                    bqk[s0:s0 + n, bcol:bcol + 1])
                        pending = (qk, list(_heads_done(jt)))
                do_rope(*pending)

            if tt == 0:
                # bulk loads deferred so the first projection starts early
                nc.sync.dma_start(out=wv_sb, in_=wv_d[:, :])
                nc.sync.dma_start(out=masks, in_=masks_d[:, :])
                for jt in range(NJT):
                    nc.sync.dma_start(out=owt[jt], in_=ow_d[jt])

            if cfg.get("dump") and tt == 0:
                nc.sync.dma_start(out=dq_d[:, :], in_=qT[0])

            # ---- v projection (natural layout + ones col) ----
            with nc.named_scope(f"t{tt}_v"), ExitStack() as pstk:
                Pv = pstk.enter_context(
                    tc.tile_pool(name=f"psv{tt}", bufs=1, space="PSUM"))
                jw = NH2 * HD // 2
                for pi in range(2):
                    pvs = [[Pv.tile([128, NH2 * HD // 2], F32,
                                    tag=f"pv{si}_{jh}",
                                    name=f"pv{tt}_{pi}_{si}_{jh}")
                            for jh in range(2)] for si in range(2)]
                    for kt in range(KT2 + 1):
                        for si in range(2):
                            stb = pi * 2 + si
                            lhsT = xt3[:, kt, stb * 128:(stb + 1) * 128]
                            for jh in range(2):
                                nc.tensor.matmul(
                                    pvs[si][jh], lhsT,
                                    wv3[:, kt, jh * jw:(jh + 1) * jw],
                                    start=(kt == 0), stop=(kt == KT2))
                    for si in range(2):
                        st = tt * 4 + pi * 2 + si
                        for h in range(NH2):
                            src = pvs[si][h // 4]
                            c0 = (h % 4) * HD
                            nc.any.tensor_copy(vA3[h][:, st, 0:HD],
                                               src[:, c0:c0 + HD])

            # ---- attention (q0 = tt) ----
            ctxt = [ctxpool.tile([128, TB], BF, tag=f"c{jt}",
                                name=f"ctx{tt}_{jt}")
                    for jt in range(NJT)]
            with nc.named_scope(f"t{tt}_at"), ExitStack() as pstk:
                Ps = pstk.enter_context(
                    tc.tile_pool(name=f"pss{tt}", bufs=5, space="PSUM"))
                Pc = pstk.enter_context(
                    tc.tile_pool(name=f"psc{tt}", bufs=2, space="PSUM"))
                VC = 97
                PIPE = 4
                nkt = 4 * (tt + 1)
                pctxs = {}

                def scores_pv(h, midfill=None):
                    pctx = Pc.tile([VC, TB], F32, tag="pc",
                                   name=f"pctx{tt}_{h}")
                    pctxs[h] = pctx
                    pts = {}
                    for kt in range(nkt + PIPE):
                        if midfill is not None and kt == nkt // 2:
                            midfill()
                        if kt < nkt:
                            od = max(kt * 128 - tt * TB, 0)
                            pss = Ps.tile([128, TB], F32, tag="ps",
                                          name=f"pss{tt}_{h}_{kt}")
                            nc.tensor.matmul(
                                pss[:, od:TB],
                                kT[h][:, kt * 128:(kt + 1) * 128],
                                qT[h][:, od:TB], start=True, stop=True)
                            pT = ptpool.tile([128, TB], BF, tag="pt",
                                             name=f"pT{tt}_{h}_{kt}")
                            nc.scalar.activation(pT[:, od:TB],
                                                 pss[:, od:TB], EXP)
                            if od > 0 or kt * 128 == tt * TB:
                                oi = od // 128
                                nc.vector.tensor_mul(
                                    pT[:, od:TB], pT[:, od:TB],
                                    masks[:, oi * TB + od:(oi + 1) * TB])
                            pts[kt] = (pT, od)
                        j = kt - PIPE
                        if 0 <= j:
                            pT, od = pts.pop(j)
                            nc.tensor.matmul(pctx[:, od:TB],
                                             vA3[h][:, j, :],
                                             pT[:, od:TB],
                                             start=(j == 0),
                                             stop=(j == nkt - 1))

                def normalize(h):
                    pctx = pctxs.pop(h)
                    denf = rdpool.tile([1, TB], F32, tag="df")
                    nc.any.tensor_copy(denf, pctx[VC - 1:VC, :])
                    rden = rdpool.tile([1, TB], F32, tag="rd")
                    nc.vector.reciprocal_approx_fast(rden, denf)
                    rb = rbpool.tile([HDP, TB], F32, tag="rb")
                    nc.gpsimd.partition_broadcast(rb, rden)
                    if cfg.get("dump") and tt == 0 and h == 0:
                        nc.sync.dma_start(out=drb_d[:, :], in_=rb)
                        nc.sync.dma_start(out=dpd_d[:, :], in_=rden)
                    ctxh = chpool.tile([HDP, TB], BF, tag="ch")
                    nc.vector.tensor_mul(ctxh, pctx[0:HDP, :], rb)
                    g0 = h * HDP
                    r = 0
                    while r < HDP:
                        jt, jr = (g0 + r) // 128, (g0 + r) % 128
                        n = min(HDP - r, 128 - jr)
                        for (s0, d0, cn) in _part_chunks(r, jr, n):
                            nc.any.tensor_copy(ctxt[jt][d0:d0 + cn, :],
                                               ctxh[s0:s0 + cn, :])
                        r += n

                def fill_one():
                    if pending_out is not None and pending_out[1]:
                        pending_out[0](pending_out[1].pop(0))

                for h in range(NH2):
                    scores_pv(h, midfill=fill_one if nkt >= 8 else None)
                    if h > 0:
                        normalize(h - 1)
                    if pending_out is not None:
                        emit_fn, rem = pending_out
                        for dt_i in rem[:2]:
                            emit_fn(dt_i)
                        del rem[:2]
                normalize(NH2 - 1)
                if pending_out is not None:
                    emit_fn, rem = pending_out
                    for dt_i in rem:
                        emit_fn(dt_i)
                    pending_out = None

            if cfg.get("dump") and tt == 0:
                nc.sync.dma_start(out=dctx_d[:, :], in_=ctxt[0])

            # ---- out projection: deferred into the next tile's attention
            pending_out = [make_out_emitter(ctxt, win, tt),
                           list(range(D // 128))]

        with nc.named_scope("t3_out"):
            emit_fn, rem = pending_out
            for dt_i in rem:
                emit_fn(dt_i)
            pending_out = None

        if cfg.get("dump"):
            nc.sync.dma_start(out=dk_d[:, :], in_=kT[0])
            nc.sync.dma_start(out=dv_d[:, :], in_=vA[0])

    nc.finalize()
    return nc


def prep_core_inputs_v2(cfg, x, wqkv_w, wqkv_b, out_w, core, _cache={}):
    import ml_dtypes
    BF_NP = ml_dtypes.bfloat16

    bat = core // 4
    hg = core % 4
    rows = np.arange(hg * NH2 * HD, (hg + 1) * NH2 * HD)
    scale = np.float32(1.0 / np.sqrt(HD))

    wq, bq = wqkv_w[rows], wqkv_b[rows]
    wk, bk = wqkv_w[D + rows] * scale, wqkv_b[D + rows] * scale
    wv, bv = wqkv_w[2 * D + rows], wqkv_b[2 * D + rows]

    def pad96(w):
        """[640, N] head-major -> [768, N], each 80-row head padded to 96."""
        out = np.zeros((NH2 * HDP,) + w.shape[1:], np.float32)
        for h in range(NH2):
            out[h * HDP:h * HDP + HD] = w[h * HD:(h + 1) * HD]
        return out

    xkey = ("x", bat)
    if xkey not in _cache:
        xa = np.zeros(((KT2 + 1) * 128, S), np.float32)
        xa[:D] = x[bat].T
        xa[D] = 1.0
        _cache[xkey] = np.ascontiguousarray(
            xa.reshape(KT2 + 1, 128, NTT, TB).transpose(2, 1, 0, 3)
        ).reshape(NTT, 128, (KT2 + 1) * TB).astype(BF_NP)
    xt = _cache[xkey]

    def qk_tiles(w):
        t = pad96(w).reshape(NJT, 128, KT2, 128)
        return np.ascontiguousarray(t.transpose(0, 3, 2, 1)).reshape(
            NJT, 128, KT2 * 128)

    wqk = np.stack([qk_tiles(wq), qk_tiles(wk)]).astype(BF_NP)
    bqk = np.stack([pad96(bq).reshape(NJT, 128),
                    pad96(bk).reshape(NJT, 128)])
    bqk = np.ascontiguousarray(bqk.transpose(2, 0, 1)).reshape(
        128, 2 * NJT).astype(np.float32)

    wva = np.zeros(((KT2 + 1) * 128, NH2 * HD), np.float32)
    wva[:D] = wv.T
    wva[D] = bv
    wv_t = np.ascontiguousarray(
        wva.reshape(KT2 + 1, 128, NH2 * HD).transpose(1, 0, 2)).reshape(
        128, (KT2 + 1) * NH2 * HD).astype(BF_NP)

    ow = np.ascontiguousarray(pad96(out_w[:, rows].T)).reshape(
        NJT, 128, D).astype(BF_NP)

    ckey = "cs"
    if ckey not in _cache:
        inv = 1.0 / (ROPE_BASE ** (np.arange(RD // 2, dtype=np.float32)
                                   / (RD // 2)))
        fr = np.outer(np.arange(S, dtype=np.float32), inv)
        cosT = np.cos(fr).T.astype(np.float32)
        sinT = np.sin(fr).T.astype(np.float32)
        C = np.concatenate([cosT, cosT], 0)
        Sp = np.concatenate([-sinT, sinT], 0)
        _cache[ckey] = np.ascontiguousarray(
            np.concatenate([C, Sp], 0)).astype(BF_NP)

        km = np.arange(128)[:, None]
        qm = np.arange(TB)[None, :]
        _cache["masks"] = np.ascontiguousarray(np.concatenate(
            [(qm >= oi * 128 + km).astype(np.float32)
             for oi in range(TB // 128)], axis=1)).astype(BF_NP)

        P32 = np.zeros((RD, RD), np.float32)
        for m in range(RD):
            P32[(m + RD // 2) % RD, m] = 1.0
        _cache["p32"] = P32.astype(BF_NP)

    return {
        "xt": xt, "wqk": wqk, "bqk": bqk, "wv": wv_t, "ow": ow,
        "cs": _cache[ckey], "masks": _cache["masks"], "p32": _cache["p32"],
    }


def finish_output_v2(res, out_b):
    y = np.zeros((B, S, D), np.float32)
    for bi in range(B):
        acc = np.zeros((D, S), np.float64)
        for c in range(4 * bi, 4 * bi + 4):
            acc += res.results[c]["y"]
        y[bi] = acc.T.astype(np.float32) + out_b[None, :]
    return y


def build_program(cfg):
    s, d, nh, hd, rd = cfg["s"], cfg["d"], cfg["nh"], cfg["hd"], cfg["rd"]
    qb, n_st, n_kt = cfg["qb"], cfg["n_st"], cfg["n_kt"]
    nqb, ndiag, n_dt, jq = cfg["nqb"], cfg["ndiag"], cfg["n_dt"], cfg["jq"]
    nb = cfg["b"]
    DT = F32 if cfg["dt_mm"] == "f32" else F32R
    rh = rd // 2

    nc = bacc.Bacc(None, debug=False)

    xs_d = [
        nc.declare_dram_parameter(f"xs_b{b}", [n_st, 128, n_kt * 128], DT,
                                  isOutput=False)
        for b in range(nb)
    ]
    wqk_d = nc.declare_dram_parameter("wqk", [128, n_kt, 2 * jq], DT,
                                      isOutput=False)
    wv_d = nc.declare_dram_parameter("wv", [128, n_kt, jq], DT, isOutput=False)
    outw_d = nc.declare_dram_parameter("outw", [nh, hd, d], DT, isOutput=False)
    cos_d = nc.declare_dram_parameter("cosN", [128, n_st * rh], F32,
                                      isOutput=False)
    sin_d = nc.declare_dram_parameter("sinN", [128, n_st * rh], F32,
                                      isOutput=False)
    onecol = ((hd + 31) // 32) * 32  # 32-aligned ones column in v_aug
    mask_d = nc.declare_dram_parameter("masks", [128, ndiag * qb], DT,
                                       isOutput=False)
    y_d = [
        nc.declare_dram_parameter(f"y_b{b}", [d, s], F32, isOutput=True)
        for b in range(nb)
    ]
    if cfg.get("dump"):
        dq_d = nc.declare_dram_parameter("dump_q", [hd, s], F32, isOutput=True)
        dk_d = nc.declare_dram_parameter("dump_k", [hd, s], F32, isOutput=True)
        dv_d = nc.declare_dram_parameter("dump_v", [n_st, 128, onecol + 1], F32,
                                         isOutput=True)
        dc_d = nc.declare_dram_parameter("dump_c", [hd, s], F32, isOutput=True)
        dcos_d = nc.declare_dram_parameter("dump_cos", [128, n_st * rh], F32,
                                           isOutput=True)

    with tile.TileContext(nc) as tc, ExitStack() as top:
        glob = top.enter_context(tc.tile_pool(name="glob", bufs=1))
        identf = glob.tile([128, 128], F32)
        make_identity(nc, identf)
        if DT is F32:
            ident = identf
        else:
            ident = glob.tile([128, 128], DT)
            nc.vector.tensor_copy(ident, identf)
        vpad = glob.tile([128, onecol + 1 - hd], F32)
        nc.vector.memset(vpad, 0.0)
        nc.vector.memset(vpad[:, onecol - hd:onecol + 1 - hd], 1.0)
        ones1 = glob.tile([1, hd], F32)
        nc.vector.memset(ones1, 1.0)
        cosN = glob.tile([128, n_st * rh], F32)
        nc.sync.dma_start(out=cosN, in_=cos_d[:, :])
        sinN = glob.tile([128, n_st * rh], F32)
        nc.sync.dma_start(out=sinN, in_=sin_d[:, :])
        masks = glob.tile([128, ndiag * qb], DT)
        nc.sync.dma_start(out=masks, in_=mask_d[:, :])

        for b in range(nb):
            with ExitStack() as bstk:
                qt_pool = bstk.enter_context(
                    tc.tile_pool(name=f"qt{b}", bufs=1))
                qT = [qt_pool.tile([hd, s], DT, tag=f"q{h}", name=f"qT{b}_{h}") for h in range(nh)]
                kT = [qt_pool.tile([hd, s], DT, tag=f"k{h}", name=f"kT{b}_{h}") for h in range(nh)]

                # ---- phase A1: q,k projection + transpose ----
                with ExitStack() as a1:
                    wp = a1.enter_context(tc.tile_pool(name=f"w1_{b}", bufs=1))
                    wqk = wp.tile([128, n_kt, 2 * jq], DT)
                    nc.sync.dma_start(out=wqk, in_=wqk_d[:, :, :])
                    xsp = a1.enter_context(tc.tile_pool(name=f"xs1_{b}", bufs=2))
                    qkn = a1.enter_context(tc.tile_pool(name=f"qkn{b}", bufs=2))
                    psA = a1.enter_context(
                        tc.tile_pool(name=f"psA{b}", bufs=2, space="PSUM"))
                    psT = a1.enter_context(
                        tc.tile_pool(name=f"psT{b}", bufs=2, space="PSUM"))
                    rtp = a1.enter_context(tc.tile_pool(name=f"rt{b}", bufs=2))
                    for st in range(n_st):
                        xs = xsp.tile([128, n_kt * 128], DT, tag="xs")
                        nc.sync.dma_start(out=xs, in_=xs_d[b][st])
                        xs3 = xs.rearrange("p (t c) -> p t c", t=n_kt)
                        stage = qkn.tile([128, 2 * jq], DT, tag="qkn")
                        for blk in range(2):
                            ps = psA.tile([128, jq], F32, tag="ps")
                            for kt in range(n_kt):
                                nc.tensor.matmul(
                                    ps,
                                    xs3[:, kt, :],
                                    wqk[:, kt, blk * jq:(blk + 1) * jq],
                                    start=(kt == 0),
                                    stop=(kt == n_kt - 1),
                                )
                            nc.scalar.copy(stage[:, blk * jq:(blk + 1) * jq], ps)
                        cN = cosN[:, st * rh:(st + 1) * rh]
                        sN = sinN[:, st * rh:(st + 1) * rh]
                        for h in range(nh):
                            for qk in range(2):
                                base = qk * jq + h * hd
                                t1 = stage[:, base:base + rh]
                                t2 = stage[:, base + rh:base + rd]
                                ta = rtp.tile([128, rh], F32, tag="ta")
                                nc.vector.tensor_mul(ta, t1, cN)
                                tb = rtp.tile([128, rh], F32, tag="tb")
                                nc.vector.tensor_mul(tb, t2, sN)
                                tg = rtp.tile([128, rh], F32, tag="tg")
                                nc.vector.tensor_mul(tg, t1, sN)
                                td = rtp.tile([128, rh], F32, tag="td")
                                nc.vector.tensor_mul(td, t2, cN)
                                nc.vector.tensor_sub(t1, ta, tb)
                                nc.vector.tensor_add(t2, tg, td)
                        for h in range(nh):
                            for qk, dstT in ((0, qT), (1, kT)):
                                pt = psT.tile([hd, 128], DT, tag="pt")
                                nc.tensor.transpose(
                                    pt,
                                    stage[:, qk * jq + h * hd:
                                          qk * jq + (h + 1) * hd],
                                    ident,
                                )
                                nc.vector.tensor_copy(
                                    dstT[h][:, st * 128:(st + 1) * 128], pt)

                # ---- phase A2: v projection (natural + ones col) ----
                vp = bstk.enter_context(tc.tile_pool(name=f"v{b}", bufs=1))
                vA = [
                    [vp.tile([128, onecol + 1], DT, tag=f"v{h}_{st}",
                             name=f"vA{b}_{h}_{st}")
                     for st in range(n_st)]
                    for h in range(nh)
                ]
                with ExitStack() as a2:
                    wp2 = a2.enter_context(tc.tile_pool(name=f"w2_{b}", bufs=1))
                    wv = wp2.tile([128, n_kt, jq], DT)
                    nc.sync.dma_start(out=wv, in_=wv_d[:, :, :])
                    xsp2 = a2.enter_context(tc.tile_pool(name=f"xs2_{b}", bufs=2))
                    psA2 = a2.enter_context(
                        tc.tile_pool(name=f"psA2{b}", bufs=2, space="PSUM"))
                    for st in range(n_st):
                        xs = xsp2.tile([128, n_kt * 128], DT, tag="xs")
                        nc.sync.dma_start(out=xs, in_=xs_d[b][st])
                        xs3 = xs.rearrange("p (t c) -> p t c", t=n_kt)
                        ps = psA2.tile([128, jq], F32, tag="ps")
                        for kt in range(n_kt):
                            nc.tensor.matmul(
                                ps,
                                xs3[:, kt, :],
                                wv[:, kt, :],
                                start=(kt == 0),
                                stop=(kt == n_kt - 1),
                            )
                        for h in range(nh):
                            nc.scalar.copy(
                                vA[h][st][:, 0:hd],
                                ps[:, h * hd:(h + 1) * hd])
                            nc.vector.tensor_copy(
                                vA[h][st][:, hd:onecol + 1], vpad)

                if cfg.get("dump") and b == 0:
                    nc.sync.dma_start(out=dcos_d[:, :], in_=cosN)
                    nc.sync.dma_start(out=dq_d[:, :], in_=qT[0])
                    nc.sync.dma_start(out=dk_d[:, :], in_=kT[0])
                    for st in range(n_st):
                        nc.sync.dma_start(out=dv_d[st], in_=vA[0][st])

                # ---- phase C: attention ----
                ctx_pool = bstk.enter_context(tc.tile_pool(name=f"ctx{b}", bufs=1))
                ctxT = [ctx_pool.tile([hd, s], DT, tag=f"c{h}", name=f"ctxT{b}_{h}") for h in range(nh)]
                with ExitStack() as cstk:
                    pp = cstk.enter_context(tc.tile_pool(name=f"pT{b}", bufs=3))
                    rp2 = cstk.enter_context(tc.tile_pool(name=f"rr{b}", bufs=2))
                    psS = cstk.enter_context(
                        tc.tile_pool(name=f"psS{b}", bufs=2, space="PSUM"))
                    psC = cstk.enter_context(
                        tc.tile_pool(name=f"psC{b}", bufs=2, space="PSUM"))
                    psB = cstk.enter_context(
                        tc.tile_pool(name=f"psB{b}", bufs=2, space="PSUM"))
                    for h in range(nh):
                        for q0 in range(nqb):
                            nkt_q = (q0 + 1) * qb // 128
                            pctx = psC.tile([onecol + 1, qb], F32, tag="pc")
                            for kt in range(nkt_q):
                                pss = psS.tile([128, qb], F32, tag="ps")
                                nc.tensor.matmul(
                                    pss,
                                    kT[h][:, kt * 128:(kt + 1) * 128],
                                    qT[h][:, q0 * qb:(q0 + 1) * qb],
                                    start=True, stop=True,
                                )
                                pT = pp.tile([128, qb], DT, tag="p")
                                nc.scalar.activation(
                                    pT, pss, mybir.ActivationFunctionType.Exp)
                                od = kt * 128 - q0 * qb
                                if od >= 0:
                                    oi = od // 128
                                    nc.vector.tensor_mul(
                                        pT, pT,
                                        masks[:, oi * qb:(oi + 1) * qb])
                                nc.tensor.matmul(
                                    pctx, vA[h][kt], pT,
                                    start=(kt == 0), stop=(kt == nkt_q - 1),
                                )
                            rden = rp2.tile([1, qb], F32, tag="rd")
                            nc.vector.reciprocal(rden, pctx[onecol:onecol + 1, :])
                            pbc = psB.tile([hd, qb], F32, tag="bc")
                            nc.tensor.matmul(pbc, ones1, rden,
                                             start=True, stop=True)
                            rb = rp2.tile([hd, qb], F32, tag="rb")
                            nc.scalar.copy(rb, pbc)
                            nc.vector.tensor_mul(
                                ctxT[h][:, q0 * qb:(q0 + 1) * qb],
                                pctx[0:hd, :], rb)

                if cfg.get("dump") and b == 0:
                    nc.sync.dma_start(out=dc_d[:, :], in_=ctxT[0])

                # ---- phase D: out projection ----
                with ExitStack() as dstk:
                    op = dstk.enter_context(tc.tile_pool(name=f"ow{b}", bufs=1))
                    ow = [op.tile([hd, d], DT, tag=f"o{h}", name=f"ow{b}_{h}") for h in range(nh)]
                    for h in range(nh):
                        nc.sync.dma_start(out=ow[h], in_=outw_d[h])
                    stp = dstk.enter_context(tc.tile_pool(name=f"st{b}", bufs=2))
                    psD = dstk.enter_context(
                        tc.tile_pool(name=f"psD{b}", bufs=2, space="PSUM"))
                    for dt_i in range(n_dt):
                        stage = stp.tile([128, s], F32, tag="y")
                        for sb in range(nqb):
                            psy = psD.tile([128, qb], F32, tag="ps")
                            for h in range(nh):
                                nc.tensor.matmul(
                                    psy,
                                    ow[h][:, dt_i * 128:(dt_i + 1) * 128],
                                    ctxT[h][:, sb * qb:(sb + 1) * qb],
                                    start=(h == 0), stop=(h == nh - 1),
                                )
                            nc.scalar.copy(stage[:, sb * qb:(sb + 1) * qb], psy)
                        nc.sync.dma_start(
                            out=y_d[b][dt_i * 128:(dt_i + 1) * 128, :],
                            in_=stage)

    nc.finalize()
    return nc


def prep_core_inputs(cfg, x, wqkv_w, wqkv_b, out_w, core):
    s, d, nh, hd, rd = cfg["s"], cfg["d"], cfg["nh"], cfg["hd"], cfg["rd"]
    qb, n_st, n_kt, d_aug = cfg["qb"], cfg["n_st"], cfg["n_kt"], cfg["d_aug"]
    ndiag, jq = cfg["ndiag"], cfg["jq"]
    nb = cfg["b"]
    rh = rd // 2
    rnd = rne11 if cfg["dt_mm"] == "f32r" else (lambda a: np.asarray(a, np.float32))

    heads = range(core * nh, (core + 1) * nh)
    rows = np.concatenate([np.arange(h * hd, (h + 1) * hd) for h in heads])
    scale = np.float32(1.0 / np.sqrt(hd))

    wq = wqkv_w[rows, :]
    bq = wqkv_b[rows]
    wk = wqkv_w[d + rows, :] * scale
    bk = wqkv_b[d + rows] * scale
    wv = wqkv_w[2 * d + rows, :]
    bv = wqkv_b[2 * d + rows]

    def wt_tiles(w, bias):
        # [d_aug, J] with row d = bias, rows > d zero -> [128, n_kt, J]
        j = w.shape[0]
        wa = np.zeros((d_aug, j), np.float32)
        wa[:d] = w.T
        wa[d] = bias
        return np.ascontiguousarray(
            wa.reshape(n_kt, 128, j).transpose(1, 0, 2))

    wqk_arr = rnd(np.concatenate([wt_tiles(wq, bq), wt_tiles(wk, bk)], axis=2))
    wv_arr = rnd(wt_tiles(wv, bv))

    outw = np.ascontiguousarray(out_w[:, rows].T.reshape(nh, hd, d))
    outw_arr = rnd(outw)

    inv_freq = 1.0 / (ROPE_BASE ** (np.arange(0, rd, 2, dtype=np.float32) / rd))
    t = np.arange(s, dtype=np.float32)
    freqs = np.outer(t, inv_freq)
    n_st_ = cfg["n_st"]
    cos_arr = np.ascontiguousarray(
        np.cos(freqs).astype(np.float32).reshape(n_st_, 128, rh)
        .transpose(1, 0, 2).reshape(128, n_st_ * rh))
    sin_arr = np.ascontiguousarray(
        np.sin(freqs).astype(np.float32).reshape(n_st_, 128, rh)
        .transpose(1, 0, 2).reshape(128, n_st_ * rh))

    km = np.arange(128)[:, None]
    qm = np.arange(qb)[None, :]
    mask_arr = np.concatenate(
        [(qm >= i * 128 + km).astype(np.float32) for i in range(ndiag)],
        axis=1)
    mask_arr = np.ascontiguousarray(mask_arr)

    in_map = {
        "wqk": wqk_arr, "wv": wv_arr, "outw": outw_arr,
        "cosN": cos_arr, "sinN": sin_arr, "masks": mask_arr,
    }
    for bi in range(nb):
        xa = np.zeros((d_aug, s), np.float32)
        xa[:d] = x[bi].T
        xa[d] = 1.0
        in_map[f"xs_b{bi}"] = rnd(
            np.ascontiguousarray(
                xa.reshape(n_kt, 128, n_st, 128).transpose(2, 1, 0, 3)
            ).reshape(n_st, 128, n_kt * 128))
    return in_map


_CACHE = {}


def finish_output(cfg, res, out_b):
    if cfg.get("ver") == 2:
        return finish_output_v2(res, out_b)
    nb, d, s = cfg["b"], cfg["d"], cfg["s"]
    y = np.zeros((nb, s, d), np.float32)
    for bi in range(nb):
        acc = np.zeros((d, s), np.float64)
        for c in range(N_CORES):
            acc += res.results[c][f"y_b{bi}"]
        y[bi] = acc.T.astype(np.float32) + out_b[None, :]
    return y


def build_any(cfg):
    if cfg.get("ver") == 2:
        return build_program_v2(cfg)
    return build_program(cfg)


def prep_any(cfg, x, wqkv_w, wqkv_b, out_w, core):
    if cfg.get("ver") == 2:
        return prep_core_inputs_v2(cfg, x, wqkv_w, wqkv_b, out_w, core)
    return prep_core_inputs(cfg, x, wqkv_w, wqkv_b, out_w, core)


def run_mha(cfg, x, wqkv_w, wqkv_b, out_w, out_b, trace=False):
    key = tuple(sorted(cfg.items()))
    if key not in _CACHE:
        _CACHE[key] = build_any(cfg)
    nc = _CACHE[key]
    in_maps = [
        prep_any(cfg, x, wqkv_w, wqkv_b, out_w, c)
        for c in range(N_CORES)
    ]
    res = run_bass_kernel_spmd(nc, in_maps, core_ids=list(range(N_CORES)),
                               trace=trace)
    return finish_output(cfg, res, out_b), res


def default_cfg():
    ver = os.environ.get("KMHA_VER", "2")
    if ver == "2":
        return make_cfg_v2()
    return make_cfg(dt_mm=os.environ.get("KMHA_DT", "f32"))


def kernel(x, wqkv_w, wqkv_b, out_w, out_b):
    cfg = default_cfg()
    y, _ = run_mha(cfg, np.asarray(x, np.float32), np.asarray(wqkv_w, np.float32),
                   np.asarray(wqkv_b, np.float32), np.asarray(out_w, np.float32),
                   np.asarray(out_b, np.float32))
    return y



# revision 40
# speedup vs baseline: 1.0379x; 1.0379x over previous
"""MHA Trainium2 kernel, 8 NeuronCores.

Problem: B=2, S=2048, D=2560, H=32 heads, HD=80, partial rotary RD=32,
causal attention, fp32 I/O.

Active path (v2, ~628us HW): hybrid sharding -- cores 0-3 take batch 0,
cores 4-7 batch 1; 8 heads per core. All matmul operands bf16 (PE runs
1 cycle/row, same as f32r, but half the DMA/SBUF), fp32 PSUM accumulate.

Per 512-token block, emitted interleaved:
  qk:  transposed-direct projection: stationary = weight tiles, moving =
       x^T tiles, so q/k land as [head_dim, tokens] with no PE transpose.
       Heads padded 80->96 rows (engine APs need 32-aligned partition
       starts; 96 makes every head-boundary piece legal). Bias added
       during the PSUM->SBUF distribution (chunked tensor_scalar_add);
       rope = PE swap-matmul (16-row block swap) + 3 wide DVE ops,
       delayed one group so the PE never waits on the staging chain.
  v:   natural layout [tokens, 80(+zeros+ones col at 96)] from a fully
       resident wv; the ones column accumulates the softmax denominator
       for free inside the PV matmul.
  at:  scoresT = kT.T @ qT (128-row padded stationaries enable fast
       weight load), exp on ACT (bf16 out), causal mask multiply only on
       diagonal tiles, columns trimmed to the valid causal range;
       scores->exp->PV software-pipelined 3 deep. Denominator row 96 ->
       reciprocal_approx_fast (via SBUF; the custom DVE op misreads at
       PSUM partition offsets) -> gpsimd partition_broadcast -> DVE
       normalize into packed 128-row ctx tiles.
  out: y^T partials [d, tokens], deferred one block and interleaved into
       the next block's attention so independent out matmuls fill the
       exp-latency stalls (keeps the PE HAM clock gate at 2.4 GHz).
Host: per batch, sum the 4 cores' fp32 partials, transpose, add bias.
"""
import sys
import os

sys.path.insert(0, "/opt/trn_rl_repo")

import numpy as np
from contextlib import ExitStack

import concourse.bacc as bacc
import concourse.tile as tile
import concourse.mybir as mybir
from concourse.bass_utils import run_bass_kernel_spmd
from concourse.masks import make_identity

F32 = mybir.dt.float32
F32R = mybir.dt.float32r

B, S, D = 2, 2048, 2560
H, HD = 32, 80
RD = 32
ROPE_BASE = 10000.0
N_CORES = 8


def rne11(x):
    """Round-to-nearest-even to 11 mantissa bits (matches HW f32r rounding)."""
    xi = np.ascontiguousarray(x, dtype=np.float32).view(np.uint32).astype(np.uint64)
    shift = np.uint64(12)
    bias = np.uint64((1 << 11) - 1)
    lsb = (xi >> shift) & np.uint64(1)
    xi = (xi + bias + lsb) >> shift << shift
    return xi.astype(np.uint32).view(np.float32)


def make_cfg(s=S, d=D, nh=H // N_CORES, hd=HD, rd=RD, b=B, qb=512, dt_mm="f32"):
    cfg = dict(s=s, d=d, nh=nh, hd=hd, rd=rd, b=b, qb=qb, dt_mm=dt_mm)
    cfg["n_st"] = s // 128
    cfg["n_kt"] = (d + 1 + 127) // 128
    cfg["d_aug"] = cfg["n_kt"] * 128
    cfg["nqb"] = s // qb
    cfg["ndiag"] = qb // 128
    cfg["n_dt"] = d // 128
    cfg["jq"] = nh * hd
    return cfg


# ---------------------------------------------------------------------------
# v2: batch x head-group hybrid sharding, bf16 operands, transposed-direct
# q/k projection, interleaved per-512-token-block emission.
# ---------------------------------------------------------------------------

BF = mybir.dt.bfloat16
NH2 = 8          # heads per core
TB = 512         # token block
NTT = S // TB    # 4 token tiles
KT2 = D // 128   # 20 contraction tiles for q,k
HDP = 96         # head rows padded to 96 (32-aligned piece boundaries)
NJT = NH2 * HDP // 128  # 6 j-tiles of 128 in padded j-space


def make_cfg_v2():
    return dict(ver=2)


def _qk_pieces(jt):
    """Rows of j-tile jt split at (padded) head boundaries: (r0, r1, h, hr0).

    All boundaries are multiples of 32 since HDP % 32 == 0.
    """
    out = []
    r = 0
    while r < 128:
        j = jt * 128 + r
        h, hr = j // HDP, j % HDP
        r1 = min(128, r + (HDP - hr))
        out.append((r, r1, h, hr))
        r = r1
    return out


def _heads_done(jt):
    """Heads fully staged after j-tile jt."""
    lo = (128 * jt) // HDP if jt > 0 else 0
    hi = (128 * (jt + 1)) // HDP
    return range(lo, hi)


def _part_chunks(src0, dst0, n):
    """Split a partition-range copy into engine-legal chunks.

    Engine APs must be naturally aligned in the partition dim: start 0 is
    unrestricted, start 64 allows <=64 rows, starts 32/96 allow <=32.
    """
    def maxsz(x):
        if x == 0:
            return 128
        return 64 if x % 64 == 0 else 32

    out = []
    r = 0
    while r < n:
        s, d = src0 + r, dst0 + r
        sz = min(maxsz(s), maxsz(d), n - r)
        out.append((s, d, sz))
        r += sz
    return out


def build_program_v2(cfg):
    EXP = mybir.ActivationFunctionType.Exp
    nc = bacc.Bacc(None, debug=False)

    xt_d = nc.declare_dram_parameter("xt", [NTT, 128, (KT2 + 1) * TB], BF,
                                     isOutput=False)
    wqk_d = nc.declare_dram_parameter("wqk", [2, NJT, 128, KT2 * 128], BF,
                                      isOutput=False)
    bqk_d = nc.declare_dram_parameter("bqk", [128, 2 * NJT], F32,
                                      isOutput=False)
    wv_d = nc.declare_dram_parameter("wv", [128, (KT2 + 1) * NH2 * HD], BF,
                                     isOutput=False)
    ow_d = nc.declare_dram_parameter("ow", [NJT, 128, D], BF, isOutput=False)
    cs_d = nc.declare_dram_parameter("cs", [2 * RD, S], BF, isOutput=False)
    masks_d = nc.declare_dram_parameter("masks", [128, 4 * TB], BF,
                                        isOutput=False)
    p32_d = nc.declare_dram_parameter("p32", [RD, RD], BF, isOutput=False)
    y_d = nc.declare_dram_parameter("y", [D, S], F32, isOutput=True)
    if cfg.get("dump"):
        dq_d = nc.declare_dram_parameter("dump_q", [HDP, TB], BF,
                                         isOutput=True)
        dk_d = nc.declare_dram_parameter("dump_k", [HDP, S], BF,
                                         isOutput=True)
        dv_d = nc.declare_dram_parameter("dump_v", [128, 16 * 97], BF,
                                         isOutput=True)
        dst_d = nc.declare_dram_parameter("dump_st", [128, TB], BF,
                                          isOutput=True)
        dctx_d = nc.declare_dram_parameter("dump_ctx", [128, TB], BF,
                                           isOutput=True)
        drb_d = nc.declare_dram_parameter("dump_rb", [HDP, TB], F32,
                                          isOutput=True)
        dpd_d = nc.declare_dram_parameter("dump_pd", [1, TB], F32,
                                          isOutput=True)

    with tile.TileContext(nc) as tc, ExitStack() as top:
        glob = top.enter_context(tc.tile_pool(name="glob", bufs=1))
        cs = glob.tile([2 * RD, S], BF)
        masks = glob.tile([128, 4 * TB], BF)
        bqk = glob.tile([128, 2 * NJT], F32)
        p32 = glob.tile([RD, RD], BF)
        owt = [glob.tile([128, D], BF, name=f"owt{jt}") for jt in range(NJT)]

        wv_sb = glob.tile([128, (KT2 + 1) * NH2 * HD], BF)
        wv3 = wv_sb.rearrange("p (k j) -> p k j", k=KT2 + 1)

        VC = 97  # v cols: 80 data + zeros + ones col at 96 (32-aligned)
        kT = [glob.tile([128, S], BF, name=f"kT{h}") for h in range(NH2)]
        for h in range(NH2):
            nc.vector.memset(kT[h][HDP:128, :], 0.0)
        vA = [glob.tile([128, 16 * VC], BF, name=f"vA{h}")
              for h in range(NH2)]
        vA3 = [v.rearrange("p (s c) -> p s c", s=16) for v in vA]
        for h in range(NH2):
            nc.vector.memset(vA3[h][:, :, HD:VC - 1], 0.0)
            nc.vector.memset(vA3[h][:, :, VC - 1:VC], 1.0)

        xpool = top.enter_context(tc.tile_pool(name="xs", bufs=1))
        wqkpool = top.enter_context(tc.tile_pool(name="wqks", bufs=3))
        qtpool = top.enter_context(tc.tile_pool(name="qt", bufs=1))
        rtmp = top.enter_context(tc.tile_pool(name="rtmp", bufs=2))
        ptpool = top.enter_context(tc.tile_pool(name="pt", bufs=5))
        rdpool = top.enter_context(tc.tile_pool(name="rd", bufs=1))
        rbpool = top.enter_context(tc.tile_pool(name="rb", bufs=2))
        chpool = top.enter_context(tc.tile_pool(name="ch", bufs=2))
        ctxpool = top.enter_context(tc.tile_pool(name="ctx", bufs=2))
        ypool = top.enter_context(tc.tile_pool(name="yst", bufs=2))
        Po = top.enter_context(tc.tile_pool(name="pso", bufs=2, space="PSUM"))

        def make_out_emitter(ctxt_cur, win_cur, tt_cur):
            def emit(dt_i):
                psy = Po.tile([128, TB], F32, tag="py",
                              name=f"psy{tt_cur}_{dt_i}")
                for jt in range(NJT):
                    nc.tensor.matmul(
                        psy, owt[jt][:, dt_i * 128:(dt_i + 1) * 128],
                        ctxt_cur[jt], start=(jt == 0), stop=(jt == NJT - 1))
                yt = ypool.tile([128, TB], F32, tag="y")
                nc.vector.tensor_copy(yt, psy)
                nc.sync.dma_start(
                    out=y_d[dt_i * 128:(dt_i + 1) * 128, win_cur], in_=yt)
            return emit

        pending_out = None  # [emit_fn, remaining dt list]

        for tt in range(NTT):
            win = slice(tt * TB, (tt + 1) * TB)
            xt = xpool.tile([128, (KT2 + 1) * TB], BF, tag="xt")
            for c0, c1 in ((0, 4), (4, 10), (10, 16), (16, KT2 + 1)):
                nc.sync.dma_start(out=xt[:, c0 * TB:c1 * TB],
                                  in_=xt_d[tt][:, c0 * TB:c1 * TB])
            xt3 = xt.rearrange("p (k t) -> p k t", k=KT2 + 1)

            qT = [qtpool.tile([128, TB], BF, tag=f"q{h}", name=f"qT{tt}_{h}")
                  for h in range(NH2)]
            if tt == 0:
                for h in range(NH2):
                    nc.vector.memset(qT[h][HDP:128, :], 0.0)

            # ---- q,k projection (transposed-direct) + rope ----
            with nc.named_scope(f"t{tt}_qk"), ExitStack() as pstk:
                P1 = pstk.enter_context(
                    tc.tile_pool(name=f"psqk{tt}", bufs=3, space="PSUM"))
                Psw = pstk.enter_context(
                    tc.tile_pool(name=f"psw{tt}", bufs=2, space="PSUM"))
                def do_rope(qk, heads):
                    for h in heads:
                        rot = (qT[h][0:RD, :] if qk == 0
                               else kT[h][0:RD, win])
                        sw = Psw.tile([RD, TB], F32, tag="sw")
                        nc.tensor.matmul(sw, p32, rot,
                                         start=True, stop=True)
                        tcos = rtmp.tile([RD, TB], BF, tag="tc")
                        nc.vector.tensor_mul(tcos, rot, cs[0:RD, win])
                        tsin = rtmp.tile([RD, TB], BF, tag="ts")
                        nc.vector.tensor_mul(tsin, sw, cs[RD:2 * RD, win])
                        nc.vector.tensor_add(rot, tcos, tsin)

                pending = None
                for qk in range(2):
                    for jt in range(NJT):
                        w = wqkpool.tile([128, KT2 * 128], BF, tag="wqk")
                        nc.sync.dma_start(out=w, in_=wqk_d[qk, jt])
                        if tt == 0 and qk == 0 and jt == 0:
                            nc.sync.dma_start(out=bqk, in_=bqk_d[:, :])
                            nc.sync.dma_start(out=cs, in_=cs_d[:, :])
                            nc.sync.dma_start(out=p32, in_=p32_d[:, :])
                        w3 = w.rearrange("p (k j) -> p k j", k=KT2)
                        ps = P1.tile([128, TB], F32, tag="ps")
                        for kt in range(KT2):
                            nc.tensor.matmul(ps, w3[:, kt, :], xt3[:, kt, :],
                                             start=(kt == 0),
                                             stop=(kt == KT2 - 1))
                        # rope for the previous group's completed heads --
                        # one group late so the PE never waits on the DVE
                        # staging chain
                        if pending:
                            do_rope(*pending)
                        bcol = qk * NJT + jt
                        for (r0, r1, h, hr0) in _qk_pieces(jt):
                            for (s0, d0, n) in _part_chunks(r0, hr0,
                                                            r1 - r0):
                                if qk == 0:
                                    dst = qT[h][d0:d0 + n, :]
                                else:
                                    dst = kT[h][d0:d0 + n, win]
                                nc.any.tensor_scalar_add(
                                    dst, ps[s0:s0 + n, :],
                                    bqk[s0:s0 + n, bcol:bcol + 1])
                        pending = (qk, list(_heads_done(jt)))
                do_rope(*pending)

            if tt == 0:
                # bulk loads deferred so the first projection starts early
                nc.sync.dma_start(out=wv_sb, in_=wv_d[:, :])
                nc.sync.dma_start(out=masks, in_=masks_d[:, :])
                for jt in range(NJT):
                    nc.sync.dma_start(out=owt[jt], in_=ow_d[jt])

            if cfg.get("dump") and tt == 0:
                nc.sync.dma_start(out=dq_d[:, :], in_=qT[0])

            # ---- v projection (natural layout + ones col) ----
            with nc.named_scope(f"t{tt}_v"), ExitStack() as pstk:
                Pv = pstk.enter_context(
                    tc.tile_pool(name=f"psv{tt}", bufs=1, space="PSUM"))
                jw = NH2 * HD // 2
                for pi in range(2):
                    pvs = [[Pv.tile([128, NH2 * HD // 2], F32,
                                    tag=f"pv{si}_{jh}",
                                    name=f"pv{tt}_{pi}_{si}_{jh}")
                            for jh in range(2)] for si in range(2)]
                    for kt in range(KT2 + 1):
                        for si in range(2):
                            stb = pi * 2 + si
                            lhsT = xt3[:, kt, stb * 128:(stb + 1) * 128]
                            for jh in range(2):
                                nc.tensor.matmul(
                                    pvs[si][jh], lhsT,
                                    wv3[:, kt, jh * jw:(jh + 1) * jw],
                                    start=(kt == 0), stop=(kt == KT2))
                    for si in range(2):
                        st = tt * 4 + pi * 2 + si
                        for h in range(NH2):
                            src = pvs[si][h // 4]
                            c0 = (h % 4) * HD
                            nc.any.tensor_copy(vA3[h][:, st, 0:HD],
                                               src[:, c0:c0 + HD])

            # ---- attention (q0 = tt) ----
            ctxt = [ctxpool.tile([128, TB], BF, tag=f"c{jt}",
                                name=f"ctx{tt}_{jt}")
                    for jt in range(NJT)]
            with nc.named_scope(f"t{tt}_at"), ExitStack() as pstk:
                Ps = pstk.enter_context(
                    tc.tile_pool(name=f"pss{tt}", bufs=4, space="PSUM"))
                Pc = pstk.enter_context(
                    tc.tile_pool(name=f"psc{tt}", bufs=2, space="PSUM"))
                VC = 97
                PIPE = 3
                nkt = 4 * (tt + 1)
                pctxs = {}

                def scores_pv(h):
                    pctx = Pc.tile([VC, TB], F32, tag="pc",
                                   name=f"pctx{tt}_{h}")
                    pctxs[h] = pctx
                    pts = {}
                    for kt in range(nkt + PIPE):
                        if kt < nkt:
                            od = max(kt * 128 - tt * TB, 0)
                            pss = Ps.tile([128, TB], F32, tag="ps",
                                          name=f"pss{tt}_{h}_{kt}")
                            nc.tensor.matmul(
                                pss[:, od:TB],
                                kT[h][:, kt * 128:(kt + 1) * 128],
                                qT[h][:, od:TB], start=True, stop=True)
                            pT = ptpool.tile([128, TB], BF, tag="pt",
                                             name=f"pT{tt}_{h}_{kt}")
                            nc.scalar.activation(pT[:, od:TB],
                                                 pss[:, od:TB], EXP)
                            if od > 0 or kt * 128 == tt * TB:
                                oi = od // 128
                                nc.vector.tensor_mul(
                                    pT[:, od:TB], pT[:, od:TB],
                                    masks[:, oi * TB + od:(oi + 1) * TB])
                            pts[kt] = (pT, od)
                        j = kt - PIPE
                        if 0 <= j:
                            pT, od = pts.pop(j)
                            nc.tensor.matmul(pctx[:, od:TB],
                                             vA3[h][:, j, :],
                                             pT[:, od:TB],
                                             start=(j == 0),
                                             stop=(j == nkt - 1))

                def normalize(h):
                    pctx = pctxs.pop(h)
                    denf = rdpool.tile([1, TB], F32, tag="df")
                    nc.any.tensor_copy(denf, pctx[VC - 1:VC, :])
                    rden = rdpool.tile([1, TB], F32, tag="rd")
                    nc.vector.reciprocal_approx_fast(rden, denf)
                    rb = rbpool.tile([HDP, TB], F32, tag="rb")
                    nc.gpsimd.partition_broadcast(rb, rden)
                    if cfg.get("dump") and tt == 0 and h == 0:
                        nc.sync.dma_start(out=drb_d[:, :], in_=rb)
                        nc.sync.dma_start(out=dpd_d[:, :], in_=rden)
                    ctxh = chpool.tile([HDP, TB], BF, tag="ch")
                    nc.vector.tensor_mul(ctxh, pctx[0:HDP, :], rb)
                    g0 = h * HDP
                    r = 0
                    while r < HDP:
                        jt, jr = (g0 + r) // 128, (g0 + r) % 128
                        n = min(HDP - r, 128 - jr)
                        for (s0, d0, cn) in _part_chunks(r, jr, n):
                            nc.any.tensor_copy(ctxt[jt][d0:d0 + cn, :],
                                               ctxh[s0:s0 + cn, :])
                        r += n

                for h in range(NH2):
                    scores_pv(h)
                    if h > 0:
                        normalize(h - 1)
                    if pending_out is not None:
                        emit_fn, rem = pending_out
                        for dt_i in rem[:2]:
                            emit_fn(dt_i)
                        del rem[:2]
                normalize(NH2 - 1)
                if pending_out is not None:
                    emit_fn, rem = pending_out
                    for dt_i in rem:
                        emit_fn(dt_i)
                    pending_out = None

            if cfg.get("dump") and tt == 0:
                nc.sync.dma_start(out=dctx_d[:, :], in_=ctxt[0])

            # ---- out projection: deferred into the next tile's attention
            pending_out = [make_out_emitter(ctxt, win, tt),
                           list(range(D // 128))]

        with nc.named_scope("t3_out"):
            emit_fn, rem = pending_out
            for dt_i in rem:
                emit_fn(dt_i)
            pending_out = None

        if cfg.get("dump"):
            nc.sync.dma_start(out=dk_d[:, :], in_=kT[0])
            nc.sync.dma_start(out=dv_d[:, :], in_=vA[0])

    nc.finalize()
    return nc


def prep_core_inputs_v2(cfg, x, wqkv_w, wqkv_b, out_w, core, _cache={}):
    import ml_dtypes
    BF_NP = ml_dtypes.bfloat16

    bat = core // 4
    hg = core % 4
    rows = np.arange(hg * NH2 * HD, (hg + 1) * NH2 * HD)
    scale = np.float32(1.0 / np.sqrt(HD))

    wq, bq = wqkv_w[rows], wqkv_b[rows]
    wk, bk = wqkv_w[D + rows] * scale, wqkv_b[D + rows] * scale
    wv, bv = wqkv_w[2 * D + rows], wqkv_b[2 * D + rows]

    def pad96(w):
        """[640, N] head-major -> [768, N], each 80-row head padded to 96."""
        out = np.zeros((NH2 * HDP,) + w.shape[1:], np.float32)
        for h in range(NH2):
            out[h * HDP:h * HDP + HD] = w[h * HD:(h + 1) * HD]
        return out

    xkey = ("x", bat)
    if xkey not in _cache:
        xa = np.zeros(((KT2 + 1) * 128, S), np.float32)
        xa[:D] = x[bat].T
        xa[D] = 1.0
        _cache[xkey] = np.ascontiguousarray(
            xa.reshape(KT2 + 1, 128, NTT, TB).transpose(2, 1, 0, 3)
        ).reshape(NTT, 128, (KT2 + 1) * TB).astype(BF_NP)
    xt = _cache[xkey]

    def qk_tiles(w):
        t = pad96(w).reshape(NJT, 128, KT2, 128)
        return np.ascontiguousarray(t.transpose(0, 3, 2, 1)).reshape(
            NJT, 128, KT2 * 128)

    wqk = np.stack([qk_tiles(wq), qk_tiles(wk)]).astype(BF_NP)
    bqk = np.stack([pad96(bq).reshape(NJT, 128),
                    pad96(bk).reshape(NJT, 128)])
    bqk = np.ascontiguousarray(bqk.transpose(2, 0, 1)).reshape(
        128, 2 * NJT).astype(np.float32)

    wva = np.zeros(((KT2 + 1) * 128, NH2 * HD), np.float32)
    wva[:D] = wv.T
    wva[D] = bv
    wv_t = np.ascontiguousarray(
        wva.reshape(KT2 + 1, 128, NH2 * HD).transpose(1, 0, 2)).reshape(
        128, (KT2 + 1) * NH2 * HD).astype(BF_NP)

    ow = np.ascontiguousarray(pad96(out_w[:, rows].T)).reshape(
        NJT, 128, D).astype(BF_NP)

    ckey = "cs"
    if ckey not in _cache:
        inv = 1.0 / (ROPE_BASE ** (np.arange(RD // 2, dtype=np.float32)
                                   / (RD // 2)))
        fr = np.outer(np.arange(S, dtype=np.float32), inv)
        cosT = np.cos(fr).T.astype(np.float32)
        sinT = np.sin(fr).T.astype(np.float32)
        C = np.concatenate([cosT, cosT], 0)
        Sp = np.concatenate([-sinT, sinT], 0)
        _cache[ckey] = np.ascontiguousarray(
            np.concatenate([C, Sp], 0)).astype(BF_NP)

        km = np.arange(128)[:, None]
        qm = np.arange(TB)[None, :]
        _cache["masks"] = np.ascontiguousarray(np.concatenate(
            [(qm >= oi * 128 + km).astype(np.float32)
             for oi in range(TB // 128)], axis=1)).astype(BF_NP)

        P32 = np.zeros((RD, RD), np.float32)
        for m in range(RD):
            P32[(m + RD // 2) % RD, m] = 1.0
        _cache["p32"] = P32.astype(BF_NP)

    return {
        "xt": xt, "wqk": wqk, "bqk": bqk, "wv": wv_t, "ow": ow,
        "cs": _cache[ckey], "masks": _cache["masks"], "p32": _cache["p32"],
    }


def finish_output_v2(res, out_b):
    y = np.zeros((B, S, D), np.float32)
    for bi in range(B):
        acc = np.zeros((D, S), np.float64)
        for c in range(4 * bi, 4 * bi + 4):
            acc += res.results[c]["y"]
        y[bi] = acc.T.astype(np.float32) + out_b[None, :]
    return y


def build_program(cfg):
    s, d, nh, hd, rd = cfg["s"], cfg["d"], cfg["nh"], cfg["hd"], cfg["rd"]
    qb, n_st, n_kt = cfg["qb"], cfg["n_st"], cfg["n_kt"]
    nqb, ndiag, n_dt, jq = cfg["nqb"], cfg["ndiag"], cfg["n_dt"], cfg["jq"]
    nb = cfg["b"]
    DT = F32 if cfg["dt_mm"] == "f32" else F32R
    rh = rd // 2

    nc = bacc.Bacc(None, debug=False)

    xs_d = [
        nc.declare_dram_parameter(f"xs_b{b}", [n_st, 128, n_kt * 128], DT,
                                  isOutput=False)
        for b in range(nb)
    ]
    wqk_d = nc.declare_dram_parameter("wqk", [128, n_kt, 2 * jq], DT,
                                      isOutput=False)
    wv_d = nc.declare_dram_parameter("wv", [128, n_kt, jq], DT, isOutput=False)
    outw_d = nc.declare_dram_parameter("outw", [nh, hd, d], DT, isOutput=False)
    cos_d = nc.declare_dram_parameter("cosN", [128, n_st * rh], F32,
                                      isOutput=False)
    sin_d = nc.declare_dram_parameter("sinN", [128, n_st * rh], F32,
                                      isOutput=False)
    onecol = ((hd + 31) // 32) * 32  # 32-aligned ones column in v_aug
    mask_d = nc.declare_dram_parameter("masks", [128, ndiag * qb], DT,
                                       isOutput=False)
    y_d = [
        nc.declare_dram_parameter(f"y_b{b}", [d, s], F32, isOutput=True)
        for b in range(nb)
    ]
    if cfg.get("dump"):
        dq_d = nc.declare_dram_parameter("dump_q", [hd, s], F32, isOutput=True)
        dk_d = nc.declare_dram_parameter("dump_k", [hd, s], F32, isOutput=True)
        dv_d = nc.declare_dram_parameter("dump_v", [n_st, 128, onecol + 1], F32,
                                         isOutput=True)
        dc_d = nc.declare_dram_parameter("dump_c", [hd, s], F32, isOutput=True)
        dcos_d = nc.declare_dram_parameter("dump_cos", [128, n_st * rh], F32,
                                           isOutput=True)

    with tile.TileContext(nc) as tc, ExitStack() as top:
        glob = top.enter_context(tc.tile_pool(name="glob", bufs=1))
        identf = glob.tile([128, 128], F32)
        make_identity(nc, identf)
        if DT is F32:
            ident = identf
        else:
            ident = glob.tile([128, 128], DT)
            nc.vector.tensor_copy(ident, identf)
        vpad = glob.tile([128, onecol + 1 - hd], F32)
        nc.vector.memset(vpad, 0.0)
        nc.vector.memset(vpad[:, onecol - hd:onecol + 1 - hd], 1.0)
        ones1 = glob.tile([1, hd], F32)
        nc.vector.memset(ones1, 1.0)
        cosN = glob.tile([128, n_st * rh], F32)
        nc.sync.dma_start(out=cosN, in_=cos_d[:, :])
        sinN = glob.tile([128, n_st * rh], F32)
        nc.sync.dma_start(out=sinN, in_=sin_d[:, :])
        masks = glob.tile([128, ndiag * qb], DT)
        nc.sync.dma_start(out=masks, in_=mask_d[:, :])

        for b in range(nb):
            with ExitStack() as bstk:
                qt_pool = bstk.enter_context(
                    tc.tile_pool(name=f"qt{b}", bufs=1))
                qT = [qt_pool.tile([hd, s], DT, tag=f"q{h}", name=f"qT{b}_{h}") for h in range(nh)]
                kT = [qt_pool.tile([hd, s], DT, tag=f"k{h}", name=f"kT{b}_{h}") for h in range(nh)]

                # ---- phase A1: q,k projection + transpose ----
                with ExitStack() as a1:
                    wp = a1.enter_context(tc.tile_pool(name=f"w1_{b}", bufs=1))
                    wqk = wp.tile([128, n_kt, 2 * jq], DT)
                    nc.sync.dma_start(out=wqk, in_=wqk_d[:, :, :])
                    xsp = a1.enter_context(tc.tile_pool(name=f"xs1_{b}", bufs=2))
                    qkn = a1.enter_context(tc.tile_pool(name=f"qkn{b}", bufs=2))
                    psA = a1.enter_context(
                        tc.tile_pool(name=f"psA{b}", bufs=2, space="PSUM"))
                    psT = a1.enter_context(
                        tc.tile_pool(name=f"psT{b}", bufs=2, space="PSUM"))
                    rtp = a1.enter_context(tc.tile_pool(name=f"rt{b}", bufs=2))
                    for st in range(n_st):
                        xs = xsp.tile([128, n_kt * 128], DT, tag="xs")
                        nc.sync.dma_start(out=xs, in_=xs_d[b][st])
                        xs3 = xs.rearrange("p (t c) -> p t c", t=n_kt)
                        stage = qkn.tile([128, 2 * jq], DT, tag="qkn")
                        for blk in range(2):
                            ps = psA.tile([128, jq], F32, tag="ps")
                            for kt in range(n_kt):
                                nc.tensor.matmul(
                                    ps,
                                    xs3[:, kt, :],
                                    wqk[:, kt, blk * jq:(blk + 1) * jq],
                                    start=(kt == 0),
                                    stop=(kt == n_kt - 1),
                                )
                            nc.scalar.copy(stage[:, blk * jq:(blk + 1) * jq], ps)
                        cN = cosN[:, st * rh:(st + 1) * rh]
                        sN = sinN[:, st * rh:(st + 1) * rh]
                        for h in range(nh):
                            for qk in range(2):
                                base = qk * jq + h * hd
                                t1 = stage[:, base:base + rh]
                                t2 = stage[:, base + rh:base + rd]
                                ta = rtp.tile([128, rh], F32, tag="ta")
                                nc.vector.tensor_mul(ta, t1, cN)
                                tb = rtp.tile([128, rh], F32, tag="tb")
                                nc.vector.tensor_mul(tb, t2, sN)
                                tg = rtp.tile([128, rh], F32, tag="tg")
                                nc.vector.tensor_mul(tg, t1, sN)
                                td = rtp.tile([128, rh], F32, tag="td")
                                nc.vector.tensor_mul(td, t2, cN)
                                nc.vector.tensor_sub(t1, ta, tb)
                                nc.vector.tensor_add(t2, tg, td)
                        for h in range(nh):
                            for qk, dstT in ((0, qT), (1, kT)):
                                pt = psT.tile([hd, 128], DT, tag="pt")
                                nc.tensor.transpose(
                                    pt,
                                    stage[:, qk * jq + h * hd:
                                          qk * jq + (h + 1) * hd],
                                    ident,
                                )
                                nc.vector.tensor_copy(
                                    dstT[h][:, st * 128:(st + 1) * 128], pt)

                # ---- phase A2: v projection (natural + ones col) ----
                vp = bstk.enter_context(tc.tile_pool(name=f"v{b}", bufs=1))
                vA = [
                    [vp.tile([128, onecol + 1], DT, tag=f"v{h}_{st}",
                             name=f"vA{b}_{h}_{st}")
                     for st in range(n_st)]
                    for h in range(nh)
                ]
                with ExitStack() as a2:
                    wp2 = a2.enter_context(tc.tile_pool(name=f"w2_{b}", bufs=1))
                    wv = wp2.tile([128, n_kt, jq], DT)
                    nc.sync.dma_start(out=wv, in_=wv_d[:, :, :])
                    xsp2 = a2.enter_context(tc.tile_pool(name=f"xs2_{b}", bufs=2))
                    psA2 = a2.enter_context(
                        tc.tile_pool(name=f"psA2{b}", bufs=2, space="PSUM"))
                    for st in range(n_st):
                        xs = xsp2.tile([128, n_kt * 128], DT, tag="xs")
                        nc.sync.dma_start(out=xs, in_=xs_d[b][st])
                        xs3 = xs.rearrange("p (t c) -> p t c", t=n_kt)
                        ps = psA2.tile([128, jq], F32, tag="ps")
                        for kt in range(n_kt):
                            nc.tensor.matmul(
                                ps,
                                xs3[:, kt, :],
                                wv[:, kt, :],
                                start=(kt == 0),
                                stop=(kt == n_kt - 1),
                            )
                        for h in range(nh):
                            nc.scalar.copy(
                                vA[h][st][:, 0:hd],
                                ps[:, h * hd:(h + 1) * hd])
                            nc.vector.tensor_copy(
                                vA[h][st][:, hd:onecol + 1], vpad)

                if cfg.get("dump") and b == 0:
                    nc.sync.dma_start(out=dcos_d[:, :], in_=cosN)
                    nc.sync.dma_start(out=dq_d[:, :], in_=qT[0])
                    nc.sync.dma_start(out=dk_d[:, :], in_=kT[0])
                    for st in range(n_st):
                        nc.sync.dma_start(out=dv_d[st], in_=vA[0][st])

                # ---- phase C: attention ----
                ctx_pool = bstk.enter_context(tc.tile_pool(name=f"ctx{b}", bufs=1))
                ctxT = [ctx_pool.tile([hd, s], DT, tag=f"c{h}", name=f"ctxT{b}_{h}") for h in range(nh)]
                with ExitStack() as cstk:
                    pp = cstk.enter_context(tc.tile_pool(name=f"pT{b}", bufs=3))
                    rp2 = cstk.enter_context(tc.tile_pool(name=f"rr{b}", bufs=2))
                    psS = cstk.enter_context(
                        tc.tile_pool(name=f"psS{b}", bufs=2, space="PSUM"))
                    psC = cstk.enter_context(
                        tc.tile_pool(name=f"psC{b}", bufs=2, space="PSUM"))
                    psB = cstk.enter_context(
                        tc.tile_pool(name=f"psB{b}", bufs=2, space="PSUM"))
                    for h in range(nh):
                        for q0 in range(nqb):
                            nkt_q = (q0 + 1) * qb // 128
                            pctx = psC.tile([onecol + 1, qb], F32, tag="pc")
                            for kt in range(nkt_q):
                                pss = psS.tile([128, qb], F32, tag="ps")
                                nc.tensor.matmul(
                                    pss,
                                    kT[h][:, kt * 128:(kt + 1) * 128],
                                    qT[h][:, q0 * qb:(q0 + 1) * qb],
                                    start=True, stop=True,
                                )
                                pT = pp.tile([128, qb], DT, tag="p")
                                nc.scalar.activation(
                                    pT, pss, mybir.ActivationFunctionType.Exp)
                                od = kt * 128 - q0 * qb
                                if od >= 0:
                                    oi = od // 128
                                    nc.vector.tensor_mul(
                                        pT, pT,
                                        masks[:, oi * qb:(oi + 1) * qb])
                                nc.tensor.matmul(
                                    pctx, vA[h][kt], pT,
                                    start=(kt == 0), stop=(kt == nkt_q - 1),
                                )
                            rden = rp2.tile([1, qb], F32, tag="rd")
                            nc.vector.reciprocal(rden, pctx[onecol:onecol + 1, :])
                            pbc = psB.tile([hd, qb], F32, tag="bc")
                            nc.tensor.matmul(pbc, ones1, rden,
                                             start=True, stop=True)
                            rb = rp2.tile([hd, qb], F32, tag="rb")
                            nc.scalar.copy(rb, pbc)
                            nc.vector.tensor_mul(
                                ctxT[h][:, q0 * qb:(q0 + 1) * qb],
                                pctx[0:hd, :], rb)

                if cfg.get("dump") and b == 0:
                    nc.sync.dma_start(out=dc_d[:, :], in_=ctxT[0])

                # ---- phase D: out projection ----
                with ExitStack() as dstk:
                    op = dstk.enter_context(tc.tile_pool(name=f"ow{b}", bufs=1))
                    ow = [op.tile([hd, d], DT, tag=f"o{h}", name=f"ow{b}_{h}") for h in range(nh)]
                    for h in range(nh):
                        nc.sync.dma_start(out=ow[h], in_=outw_d[h])
                    stp = dstk.enter_context(tc.tile_pool(name=f"st{b}", bufs=2))
                    psD = dstk.enter_context(
                        tc.tile_pool(name=f"psD{b}", bufs=2, space="PSUM"))
                    for dt_i in range(n_dt):
                        stage = stp.tile([128, s], F32, tag="y")
                        for sb in range(nqb):
                            psy = psD.tile([128, qb], F32, tag="ps")
                            for h in range(nh):
                                nc.tensor.matmul(
                                    psy,
                                    ow[h][:, dt_i * 128:(dt_i + 1) * 128],
                                    ctxT[h][:, sb * qb:(sb + 1) * qb],
                                    start=(h == 0), stop=(h == nh - 1),
                                )
                            nc.scalar.copy(stage[:, sb * qb:(sb + 1) * qb], psy)
                        nc.sync.dma_start(
                            out=y_d[b][dt_i * 128:(dt_i + 1) * 128, :],
                            in_=stage)

    nc.finalize()
    return nc


def prep_core_inputs(cfg, x, wqkv_w, wqkv_b, out_w, core):
    s, d, nh, hd, rd = cfg["s"], cfg["d"], cfg["nh"], cfg["hd"], cfg["rd"]
    qb, n_st, n_kt, d_aug = cfg["qb"], cfg["n_st"], cfg["n_kt"], cfg["d_aug"]
    ndiag, jq = cfg["ndiag"], cfg["jq"]
    nb = cfg["b"]
    rh = rd // 2
    rnd = rne11 if cfg["dt_mm"] == "f32r" else (lambda a: np.asarray(a, np.float32))

    heads = range(core * nh, (core + 1) * nh)
    rows = np.concatenate([np.arange(h * hd, (h + 1) * hd) for h in heads])
    scale = np.float32(1.0 / np.sqrt(hd))

    wq = wqkv_w[rows, :]
    bq = wqkv_b[rows]
    wk = wqkv_w[d + rows, :] * scale
    bk = wqkv_b[d + rows] * scale
    wv = wqkv_w[2 * d + rows, :]
    bv = wqkv_b[2 * d + rows]

    def wt_tiles(w, bias):
        # [d_aug, J] with row d = bias, rows > d zero -> [128, n_kt, J]
        j = w.shape[0]
        wa = np.zeros((d_aug, j), np.float32)
        wa[:d] = w.T
        wa[d] = bias
        return np.ascontiguousarray(
            wa.reshape(n_kt, 128, j).transpose(1, 0, 2))

    wqk_arr = rnd(np.concatenate([wt_tiles(wq, bq), wt_tiles(wk, bk)], axis=2))
    wv_arr = rnd(wt_tiles(wv, bv))

    outw = np.ascontiguousarray(out_w[:, rows].T.reshape(nh, hd, d))
    outw_arr = rnd(outw)

    inv_freq = 1.0 / (ROPE_BASE ** (np.arange(0, rd, 2, dtype=np.float32) / rd))
    t = np.arange(s, dtype=np.float32)
    freqs = np.outer(t, inv_freq)
    n_st_ = cfg["n_st"]
    cos_arr = np.ascontiguousarray(
        np.cos(freqs).astype(np.float32).reshape(n_st_, 128, rh)
        .transpose(1, 0, 2).reshape(128, n_st_ * rh))
    sin_arr = np.ascontiguousarray(
        np.sin(freqs).astype(np.float32).reshape(n_st_, 128, rh)
        .transpose(1, 0, 2).reshape(128, n_st_ * rh))

    km = np.arange(128)[:, None]
    qm = np.arange(qb)[None, :]
    mask_arr = np.concatenate(
        [(qm >= i * 128 + km).astype(np.float32) for i in range(ndiag)],
        axis=1)
    mask_arr = np.ascontiguousarray(mask_arr)

    in_map = {
        "wqk": wqk_arr, "wv": wv_arr, "outw": outw_arr,
        "cosN": cos_arr, "sinN": sin_arr, "masks": mask_arr,
    }
    for bi in range(nb):
        xa = np.zeros((d_aug, s), np.float32)
        xa[:d] = x[bi].T
        xa[d] = 1.0
        in_map[f"xs_b{bi}"] = rnd(
            np.ascontiguousarray(
                xa.reshape(n_kt, 128, n_st, 128).transpose(2, 1, 0, 3)
            ).reshape(n_st, 128, n_kt * 128))
    return in_map


_CACHE = {}


def finish_output(cfg, res, out_b):
    if cfg.get("ver") == 2:
        return finish_output_v2(res, out_b)
    nb, d, s = cfg["b"], cfg["d"], cfg["s"]
    y = np.zeros((nb, s, d), np.float32)
    for bi in range(nb):
        acc = np.zeros((d, s), np.float64)
        for c in range(N_CORES):
            acc += res.results[c][f"y_b{bi}"]
        y[bi] = acc.T.astype(np.float32) + out_b[None, :]
    return y


def build_any(cfg):
    if cfg.get("ver") == 2:
        return build_program_v2(cfg)
    return build_program(cfg)


def prep_any(cfg, x, wqkv_w, wqkv_b, out_w, core):
    if cfg.get("ver") == 2:
        return prep_core_inputs_v2(cfg, x, wqkv_w, wqkv_b, out_w, core)
    return prep_core_inputs(cfg, x, wqkv_w, wqkv_b, out_w, core)


def run_mha(cfg, x, wqkv_w, wqkv_b, out_w, out_b, trace=False):
    key = tuple(sorted(cfg.items()))
    if key not in _CACHE:
        _CACHE[key] = build_any(cfg)
    nc = _CACHE[key]
    in_maps = [
        prep_any(cfg, x, wqkv_w, wqkv_b, out_w, c)
        for c in range(N_CORES)
    ]
    res = run_bass_kernel_spmd(nc, in_maps, core_ids=list(range(N_CORES)),
                               trace=trace)
    return finish_output(cfg, res, out_b), res


def default_cfg():
    ver = os.environ.get("KMHA_VER", "2")
    if ver == "2":
        return make_cfg_v2()
    return make_cfg(dt_mm=os.environ.get("KMHA_DT", "f32"))


def kernel(x, wqkv_w, wqkv_b, out_w, out_b):
    cfg = default_cfg()
    y, _ = run_mha(cfg, np.asarray(x, np.float32), np.asarray(wqkv_w, np.float32),
                   np.asarray(wqkv_b, np.float32), np.asarray(out_w, np.float32),
                   np.asarray(out_b, np.float32))
    return y

